# revision 1
# baseline (speedup 1.0000x reference)
"""BarrierNet forward on 8 Trainium2 NeuronCores (pure batch data-parallel).

Math actually needed (x32 / x0 branches of the reference are dead code):
    h   = relu(x @ W1 + b1)                       [B, 2048]
    a   = relu(h @ W21 + b21)                     [B, 1024]
    t   = a @ W31                                 [B, 2]    (bias folded below)
    out = clip(-t + bias2, lo2, hi2)              [B, 2]
with host-folded per-channel constants
    bias2 = -(b31 + 2*om/os),  lo2 = (lo-om)/os,  hi2 = (hi-om)/os
    lo = [-(1+s3), -(1+s1)],   hi = [1+s2, 1+s0]

Device dataflow keeps features on the partition dim (x^T -> h^T -> a^T ->
x31^T) so every weight matrix is used directly as the stationary lhsT.

Perf notes (measured on trn2, 8 cores: median ~286us, best 285.3us,
run-to-run jitter +-1.5us, vs 382us baseline; rel_l2 4.63e-3):
- bf16 weights+activations: streams at the same 1 col/cycle as fp32r but
  halves LDWEIGHTS (95ns vs 187ns, fully hidden under MULT) and the W21
  DMA. rel_l2 4.6e-3 (gate 2e-2). fp8 DoubleRow would be 2x on mm2 but
  measures rel_l2 4.1e-2 — fails the gate; 2-term fp8 splits cost >= 1.0
  cyc/row, no better than bf16.
- mm1's K=8 is padded to a standard K=128 matmul by replicating x 16x
  along features with W1_rep[p, j] = W1[p % 8, j] / 16 (sum = 16
  identical groups scaled back). Uniform (128,128) tile config; the
  previous K=8 + tile_position packing forced PE tile-mode switches
  that stalled adjacent mm2 matmuls 2-3x.
- Output stays feature-major on device ([2, B_SH], contiguous 2-row DMA
  per chunk), transposed on host; a [B_SH, 2] device layout needs a
  512-descriptor scatter DMA per chunk (adds a ~38us tail).
- PE DVFS warm-up: ~10 throwaway matmuls on a zeroed scratch tile bridge
  the ~8-13us window where input DMAs are still in flight; the PE clock
  ramps 0.65 -> 1.2 -> 2.4 GHz only after ~6-8us of sustained work and
  drops back if the engine idles.
- Small input DMAs are consolidated (one f32 const blob, one packed w31,
  w1 in 4 column pieces) so the serial SP issue stream gets the 16 W21
  tiles onto the DMA queues early; mm1 of chunk c+1 is dripped between
  mm2 m-tiles so its PSUM evacuations never backlog.
PE stream floor is 1216 matmuls x 512 cols at 1 col/cycle/2.4GHz =
257us; the kernel runs ~266-268us of PE busy plus ~8us startup and
~5.5us tail/drain.
"""

import os

import numpy as np

B, N_IN, H1, H2, N_CL = 32768, 8, 2048, 1024, 2
N_CORES = 8
B_SH = B // N_CORES  # 4096 rows per core
NB = 512             # batch-chunk width (matmul free dim / PSUM bank)
N_CHUNKS = B_SH // NB
REP = 128 // N_IN    # x replication factor for the K=128 mm1
MT1 = H1 // 128      # 16 output tiles of mm1
KT2, MT2 = H1 // 128, H2 // 128  # 16 k-tiles, 8 m-tiles of mm2
KT3 = H2 // 128      # 8 k-tiles of mm3

MM_MODE = os.environ.get("BARRIER_MM_MODE", "bf16")  # fp32r | bf16 | fp32
LEAD = int(os.environ.get("BARRIER_LEAD", "1"))  # mm1 chunks in flight ahead
WARM = int(os.environ.get("BARRIER_WARM", "10"))  # PE p-state warm-up matmuls
TRACE = bool(int(os.environ.get("BARRIER_TRACE", "0")))

_CACHE = {}
last_results = None  # BassKernelResults of the most recent run (for test.py)


def _build(mode):
    from contextlib import ExitStack

    import concourse.bass as bass
    import concourse.mybir as mybir
    import concourse.tile as tile
    from concourse import bacc

    f32 = mybir.dt.float32
    if mode == "bf16":
        io_dt = mybir.dt.bfloat16
    elif mode == "fp32r":
        io_dt = mybir.dt.float32r
    else:
        io_dt = f32

    nc = bacc.Bacc("TRN2", debug=False, num_devices=N_CORES)

    xT_d = nc.dram_tensor("xT", [128, B_SH], io_dt, kind="ExternalInput").ap()
    w1_d = nc.dram_tensor("w1", [128, H1], io_dt, kind="ExternalInput").ap()
    w21_d = nc.dram_tensor("w21", [H1, H2], io_dt, kind="ExternalInput").ap()
    # w31 pre-packed on host into the SBUF layout [128, KT3*N_CL]
    w31_d = nc.dram_tensor("w31", [128, KT3 * N_CL], io_dt, kind="ExternalInput").ap()
    # all f32 constants in one blob: cols [0:16]=b1, [16:24]=b21, [24:27]=post
    cst_d = nc.dram_tensor("cst", [128, MT1 + MT2 + 3], f32, kind="ExternalInput").ap()
    out_d = nc.dram_tensor("out", [N_CL, B_SH], f32, kind="ExternalOutput").ap()

    Relu = mybir.ActivationFunctionType.Relu
    Ident = mybir.ActivationFunctionType.Identity
    add_op = mybir.AluOpType.add
    max_op = mybir.AluOpType.max
    min_op = mybir.AluOpType.min

    with tile.TileContext(nc) as tc, ExitStack() as ctx:
        const = ctx.enter_context(tc.tile_pool(name="const", bufs=1))
        wpool = ctx.enter_context(tc.tile_pool(name="w21", bufs=1))
        hpool = ctx.enter_context(tc.tile_pool(name="hT", bufs=LEAD + 1))
        apool = ctx.enter_context(tc.tile_pool(name="aT", bufs=1))
        opool = ctx.enter_context(tc.tile_pool(name="post", bufs=2))
        ps_h = ctx.enter_context(tc.tile_pool(name="ps_h", bufs=3, space="PSUM"))
        ps_a = ctx.enter_context(tc.tile_pool(name="ps_a", bufs=3, space="PSUM"))
        ps_o = ctx.enter_context(tc.tile_pool(name="ps_o", bufs=1, space="PSUM"))
        ps_w = ctx.enter_context(tc.tile_pool(name="ps_w", bufs=1, space="PSUM"))

        # PE p-state warm-up: the PE clocks 0.65 -> 1.2 -> 2.4 GHz only
        # after sustained execution, and the first ~10us of real matmuls
        # otherwise run at half speed. Stream a few throwaway matmuls on a
        # zeroed scratch tile during the window where the PE would idle
        # waiting for the first input DMAs. The memset runs on GpSimd,
        # whose sequencer comes up earliest (the tile framework requires
        # the scratch to be written before the PE reads it).
        scratch = const.tile([128, NB], io_dt)
        nc.gpsimd.memset(scratch, 0.0)
        for _ in range(WARM):
            pw = ps_w.tile([128, NB], f32, tag="warm")
            nc.tensor.matmul(
                pw, scratch[:, 0:128], scratch, start=True, stop=True
            )

        # Stationary weights / constants. DMA issue order is the critical
        # path: SP issues serially (~0.6us each) and nothing lands before
        # ~12us of NEFF startup. Issue exactly what the mm1 prologue needs
        # (chunk-0 xT, w1, biases), then the rest of xT, then the 16 W21
        # tiles so they spread across the DMA queues and finish before the
        # prologue runs dry.
        w1_sb = const.tile([128, H1], io_dt)
        xT_sb = const.tile([128, B_SH], io_dt)
        # w1 split into 4 column pieces: the chunk-0 mm1 consumes m-tiles
        # in order, so the first matmul only waits on the first piece.
        nc.sync.dma_start(out=xT_sb[:, 0:NB], in_=xT_d[:, 0:NB])
        # w1's last piece is deferred behind the w21 issues: it frees a
        # DMA-queue slot while the first matmul's inputs land, and its four
        # dependent mm1 drips fit in the PE's 4-deep dependency wait queue
        # even if it arrives late.
        for p in range(3):
            nc.sync.dma_start(
                out=w1_sb[:, p * 512 : (p + 1) * 512],
                in_=w1_d[:, p * 512 : (p + 1) * 512],
            )
        cst_sb = const.tile([128, MT1 + MT2 + 3], f32)
        nc.sync.dma_start(out=cst_sb, in_=cst_d)
        b1_sb = cst_sb[:, 0:MT1]
        b21_sb = cst_sb[:, MT1 : MT1 + MT2]
        post_sb = cst_sb[0:N_CL, MT1 + MT2 : MT1 + MT2 + 3]
        w21_t = []
        for k in range(KT2):
            t = wpool.tile([128, H2], io_dt, tag=f"w21_{k}")
            nc.sync.dma_start(out=t, in_=w21_d[k * 128 : (k + 1) * 128, :])
            w21_t.append(t)
        # xT for chunks 1-7 in two pieces: a DMA's completion semaphore
        # covers its whole region, so the chunk-1..3 mm1 drips would
        # otherwise wait for chunk 7's columns too.
        nc.sync.dma_start(out=w1_sb[:, 3 * 512 :], in_=w1_d[:, 3 * 512 :])
        nc.sync.dma_start(out=xT_sb[:, NB : 4 * NB], in_=xT_d[:, NB : 4 * NB])
        nc.sync.dma_start(out=xT_sb[:, 4 * NB :], in_=xT_d[:, 4 * NB :])
        w31_sb = const.tile([128, KT3 * N_CL], io_dt)
        nc.sync.dma_start(out=w31_sb, in_=w31_d)

        hts = {}  # chunk -> list of hT tiles

        def mm1_tile(c, m, skip_group_check=False):
            # One m-tile of hT = relu(W1_rep^T @ xT_rep + b1), K=128.
            ph = ps_h.tile([128, NB], f32)
            nc.tensor.matmul(
                ph,
                w1_sb[:, m * 128 : (m + 1) * 128],
                xT_sb[:, c * NB : (c + 1) * NB],
                start=True,
                stop=True,
                skip_group_check=skip_group_check,
            )
            ht = hpool.tile([128, NB], io_dt, tag=f"h{m}")
            if m % 2 == 0:
                nc.scalar.activation(ht, ph, Relu, bias=b1_sb[:, m : m + 1])
            else:
                nc.vector.tensor_scalar(
                    out=ht, in0=ph, scalar1=b1_sb[:, m : m + 1],
                    scalar2=0.0, op0=add_op, op1=max_op,
                )
            hts.setdefault(c, []).append(ht)

        def mm23(c, hT, fused_first=False):
            t_next = c + LEAD  # mm1 chunk interleaved into this mm23 pass
            last = c == N_CHUNKS - 1
            po = ps_o.tile([N_CL, NB], f32)
            # mm2: aT = relu(W21^T @ hT + b21)
            aT = []
            for m in range(MT2):
                pa = ps_a.tile([128, NB], f32)
                for k in range(KT2):
                    nc.tensor.matmul(
                        pa,
                        w21_t[k][:, m * 128 : (m + 1) * 128],
                        hT[k],
                        start=(k == 0),
                        stop=(k == KT2 - 1),
                    )
                    # Fused chunk-0 pass: the remaining 12 prologue mm1
                    # matmuls ride inside the m=0 chain (mm1 leads the
                    # chain's reads by 4 tiles so each evacuation lands in
                    # time). The chain's matmuls fill the PE stall slots
                    # the evac-throttled prologue otherwise leaves.
                    if fused_first and m == 0 and k + 4 < MT1:
                        mm1_tile(c, k + 4, skip_group_check=True)
                at = apool.tile([128, NB], io_dt, tag=f"a{m}")
                if m % 2 == 0:
                    nc.scalar.activation(at, pa, Relu, bias=b21_sb[:, m : m + 1])
                else:
                    nc.vector.tensor_scalar(
                        out=at, in0=pa, scalar1=b21_sb[:, m : m + 1],
                        scalar2=0.0, op0=add_op, op1=max_op,
                    )
                aT.append(at)
                # Drip two mm1 matmuls of a later chunk between mm2 m-tiles
                # so their PSUM evacuations never backlog ps_h and stall PE.
                if t_next < N_CHUNKS:
                    mm1_tile(t_next, 2 * m)
                    mm1_tile(t_next, 2 * m + 1)
            # mm3 + QP postprocess: out = clip(-t + bias2, lo2, hi2); the
            # last chunk runs it in column halves so each half's two-op
            # post chain and output DMA overlap the other half's mm3.
            v = opool.tile([N_CL, NB], f32, tag="v")
            for h0, h1 in ([(0, 256), (256, NB)] if last else [(0, NB)]):
                for k in range(KT3):
                    nc.tensor.matmul(
                        po[:, h0:h1],
                        w31_sb[:, k * N_CL : (k + 1) * N_CL],
                        aT[k][:, h0:h1],
                        start=(k == 0),
                        stop=(k == KT3 - 1),
                    )
                nc.scalar.activation(
                    v[:, h0:h1], po[:, h0:h1], Ident,
                    bias=post_sb[:, 0:1], scale=-1.0,
                )
                nc.vector.tensor_scalar(
                    out=v[:, h0:h1], in0=v[:, h0:h1],
                    scalar1=post_sb[:, 1:2], scalar2=post_sb[:, 2:3],
                    op0=max_op, op1=min_op,
                )
                nc.sync.dma_start(out=out_d[:, c * NB + h0 : c * NB + h1], in_=v[:, h0:h1])

        # Software pipeline: a 4-tile mm1 prologue, then chunk-0's pass
        # carries the remaining 12 prologue mm1s fused into its first
        # chain; each later pass carries the mm1 of chunk c+LEAD. The PE
        # stays dense while the 4MB W21 DMA streams in.
        for m in range(4):
            mm1_tile(0, m)
        if LEAD > 1:
            for c in range(1, min(LEAD, N_CHUNKS)):
                for m in range(MT1):
                    mm1_tile(c, m)
            for m in range(4, MT1):
                mm1_tile(0, m)
            fused = False
        else:
            fused = True
        for c in range(N_CHUNKS):
            mm23(c, hts[c], fused_first=(fused and c == 0))
            hts.pop(c)

    nc.compile()
    return nc


def _get_nc():
    if MM_MODE not in _CACHE:
        _CACHE[MM_MODE] = _build(MM_MODE)
    return _CACHE[MM_MODE]


def kernel(**inputs):
    global last_results
    from concourse.bass_utils import run_bass_kernel_spmd

    f32 = np.float32
    x = np.asarray(inputs["x"], f32)
    W1 = np.asarray(inputs["W1"], f32)
    b1 = np.ascontiguousarray(np.asarray(inputs["b1"], f32))
    W21 = np.asarray(inputs["W21"], f32)
    b21 = np.ascontiguousarray(np.asarray(inputs["b21"], f32))
    W31 = np.asarray(inputs["W31"], f32)
    b31 = np.asarray(inputs["b31"], f32)
    om = np.asarray(inputs["output_mean"], f32)
    os_ = np.asarray(inputs["output_std"], f32)
    s0 = np.asarray(inputs["s0"], f32)[0]
    s1 = np.asarray(inputs["s1"], f32)[0]
    s2 = np.asarray(inputs["s2"], f32)[0]
    s3 = np.asarray(inputs["s3"], f32)[0]

    lo = np.array([-(1.0 + s3), -(1.0 + s1)], f32)
    hi = np.array([1.0 + s2, 1.0 + s0], f32)
    bias2 = -(b31 + 2.0 * om / os_)
    post = np.stack([bias2, (lo - om) / os_, (hi - om) / os_], axis=1).astype(f32)
    # One f32 constant blob [128, 27]: b1 (16 cols), b21 (8), post (3).
    cst = np.zeros((128, MT1 + MT2 + 3), f32)
    cst[:, 0:MT1] = b1.reshape(MT1, 128).T
    cst[:, MT1 : MT1 + MT2] = b21.reshape(MT2, 128).T
    cst[0:N_CL, MT1 + MT2 :] = post

    if MM_MODE == "bf16":
        import ml_dtypes

        conv = lambda a: np.ascontiguousarray(a.astype(ml_dtypes.bfloat16))
    else:
        conv = lambda a: np.ascontiguousarray(a)
    # K=128 mm1 operands: W1 tiled REP times along features, scaled by
    # 1/REP (exact power of two); x replicated to match.
    w1_rep = np.tile(W1, (REP, 1)) * f32(1.0 / REP)
    # w31 packed to SBUF layout [128, KT3*N_CL]: k-tiles side by side.
    w31_pk = np.concatenate(
        [W31[k * 128 : (k + 1) * 128, :] for k in range(KT3)], axis=1
    )
    w1c, w21c, w31c = conv(w1_rep), conv(W21), conv(w31_pk)

    in_maps = []
    for c in range(N_CORES):
        xT = np.tile(x[c * B_SH : (c + 1) * B_SH].T, (REP, 1))
        in_maps.append(
            {"xT": conv(xT), "w1": w1c, "w21": w21c, "w31": w31c, "cst": cst}
        )

    nc = _get_nc()
    last_results = run_bass_kernel_spmd(
        nc, in_maps, list(range(N_CORES)), trace=TRACE
    )
    return np.ascontiguousarray(
        np.concatenate(
            [last_results.results[c]["out"].T for c in range(N_CORES)], axis=0
        ).astype(f32)
    )

